# revision 1
# baseline (speedup 1.0000x reference)
"""Graphormer attention (N=2048, D=512, H=8 heads of 64) on 8 NeuronCores.

Strategy (tensor-parallel over heads, one head per core):
  - Host slices Q/K/V/O projection weights per head, transposes x once.
  - The z-bin bias is folded in multiplicatively: the per-head bias table is
    tiny (16 entries), so host precomputes W = exp(z_table[bin(z)]) transposed
    to the kernel's [key, query] layout, shipped as bf16.
  - On device (per core): Q^T/K^T/V projections, S^T = K^T-tiles x Q^T
    (fp32, PSUM), exp on ScalarE -> bf16, P = exp(S) * W on VectorE,
    O'^T = sum_k V'[k,65] x P (65th V column = ones => row 64 of O' is the
    softmax denominator Z), then Y^T = Wo_h^T-tiles x O^T.
  - Host divides each head's partial Y by its Z, sums heads, adds bias terms.
"""

import numpy as np
import ml_dtypes
from contextlib import ExitStack

import concourse.bass as bass
import concourse.tile as tile
from concourse import bacc, mybir
from concourse import bass_utils

N = 2048
D = 512
H = 8
HD = 64
NUM_Z_BINS = 16
MAX_Z = 5.0
SCALE = HD ** -0.5
NCORES = 8
QL = 1024          # query-chunk length (PSUM budget)
QC = N // QL       # 2 query chunks
KT = N // 128      # 16 key tiles

FP32 = mybir.dt.float32
FP16 = mybir.dt.float16
BF16 = mybir.dt.bfloat16
BF16_NP = ml_dtypes.bfloat16
FP16_NP = np.float16

AF = mybir.ActivationFunctionType
OP = mybir.AluOpType

_PROGRAM_CACHE = {}


def _build_program():
    if "nc" in _PROGRAM_CACHE:
        return _PROGRAM_CACHE["nc"]

    nc = bacc.Bacc(
        "TRN2",
        target_bir_lowering=False,
        debug=False,
        enable_asserts=False,
        num_devices=NCORES,
    )

    xT = nc.dram_tensor("xT", [D, N], BF16, kind="ExternalInput").ap()
    wqk = nc.dram_tensor("wqk", [D, 128], BF16, kind="ExternalInput").ap()
    wv = nc.dram_tensor("wv", [D, HD], BF16, kind="ExternalInput").ap()
    wo = nc.dram_tensor("wo", [HD, D], FP16, kind="ExternalInput").ap()
    bqk = nc.dram_tensor("bqk", [128], FP32, kind="ExternalInput").ap()
    sclv = nc.dram_tensor("sclv", [128], FP32, kind="ExternalInput").ap()
    wt = nc.dram_tensor("wt", [N, N], FP16, kind="ExternalInput").ap()

    ypT = nc.dram_tensor("ypT", [D, N], FP16, kind="ExternalOutput").ap()
    zrow = nc.dram_tensor("zrow", [N], FP16, kind="ExternalOutput").ap()

    with tile.TileContext(nc) as tc:
        with ExitStack() as ctx:
            _emit(ctx, tc, xT, wqk, wv, wo, bqk, sclv, wt, ypT, zrow)
    nc.compile()
    _PROGRAM_CACHE["nc"] = nc
    return nc


def _emit(ctx, tc, xT, wqk, wv, wo, bqk, sclv, wt, ypT, zrow):
    nc = tc.nc
    CH = D // 128  # 4 contraction chunks of the model dim

    singles = ctx.enter_context(tc.tile_pool(name="singles", bufs=1))
    # PSUM budget is 16KB/partition (8 banks). ps_a slots are [128,1024]fp32
    # (4KB/part, 3 slots = 6 banks) shared by the qk/v/s/y phases; ps_o (2
    # banks) holds the O' accumulator.
    ps_a = ctx.enter_context(tc.tile_pool(name="ps_a", bufs=3, space="PSUM"))
    ps_o = ctx.enter_context(tc.tile_pool(name="ps_o", bufs=1, space="PSUM"))
    wpool = ctx.enter_context(tc.tile_pool(name="wpool", bufs=5))
    epool = ctx.enter_context(tc.tile_pool(name="epool", bufs=3))
    ppool = ctx.enter_context(tc.tile_pool(name="ppool", bufs=3))
    ypool = ctx.enter_context(tc.tile_pool(name="ypool", bufs=4))

    # ---- load constants -------------------------------------------------
    # column-chunked x^T: tile j holds columns [j*512,(j+1)*512) for every
    # 128-row contraction chunk, so compute can start after the first chunk.
    xr = xT.rearrange("(c p) (j n) -> j c p n", p=128, n=512)
    xT_cc = []
    for j in range(N // 512):
        t_ = singles.tile([128, CH * 512], BF16, tag=f"xTc{j}")
        for c in range(CH):
            nc.sync.dma_start(out=t_[:, c * 512:(c + 1) * 512], in_=xr[j, c])
        xT_cc.append(t_)
    # PE warm-up: dummy matmuls on never-written scratch keep the HAM busy
    # (and warm) while the x^T DMA streams in.
    scratch = singles.tile([128, 512], BF16)
    nc.vector.memset(scratch, 0.0)
    wu = ps_a.tile([128, 512], FP32, tag="big")
    for _ in range(16):
        nc.tensor.matmul(wu, lhsT=scratch[:, 0:128], rhs=scratch,
                         start=True, stop=True)
    wqk_sb = singles.tile([128, CH * 128], BF16)
    wv_sb = singles.tile([128, CH * HD], BF16)
    for c in range(CH):
        nc.sync.dma_start(out=wqk_sb[:, c * 128:(c + 1) * 128],
                          in_=wqk.rearrange("(c p) m -> c p m", p=128)[c])
        nc.sync.dma_start(out=wv_sb[:, c * HD:(c + 1) * HD],
                          in_=wv.rearrange("(c p) m -> c p m", p=128)[c])
    wo_sb = singles.tile([HD, D], FP16)
    nc.sync.dma_start(out=wo_sb, in_=wo)
    bqk_sb = singles.tile([128, 1], FP32)
    nc.sync.dma_start(out=bqk_sb, in_=bqk.rearrange("(n a) -> n a", a=1))

    # ---- fused Q^T/K^T projection: one [128,128] weight block computes
    # Q^T into PSUM rows 0-63 and K^T into rows 64-127 (full PE array).
    qT_sb = singles.tile([HD, N], BF16)
    kT_sb = singles.tile([HD, N], BF16)
    for half in range(N // 1024):
        pt = ps_a.tile([128, 1024], FP32, tag="big")
        for n in range(2):
            j = half * 2 + n
            for c in range(CH):
                nc.tensor.matmul(
                    pt[:, n * 512:(n + 1) * 512],
                    lhsT=wqk_sb[:, c * 128:(c + 1) * 128],
                    rhs=xT_cc[j][:, c * 512:(c + 1) * 512],
                    start=(c == 0),
                    stop=(c == CH - 1),
                )
        dst = slice(half * 1024, (half + 1) * 1024)
        nc.vector.tensor_scalar(kT_sb[:, dst], pt[HD:128, :], bqk_sb[HD:128, :],
                                None, OP.add)
        nc.vector.tensor_scalar(qT_sb[:, dst], pt[0:HD, :], bqk_sb[0:HD, :],
                                SCALE, OP.add, OP.mult)

    # ---- main loop: S^T -> exp -> *W -> PV, software-pipelined ----------
    # PV(t) depends on exp/mult of t; emitting S(t+1) BEFORE PV(t) keeps the
    # tensor engine's in-order queue from stalling on the ACT/DVE chain.
    # Each qc's Y projection + output DMA overlaps the next qc's loop.
    v_sb = singles.tile([128, KT * (HD + 1)], FP16)
    oT_sb = singles.tile([HD + 1, N], FP16)

    def emit_y(n2, vector_only):
        # Y^T block for query columns [n2*1024, (n2+1)*1024); when emitted in
        # the shadow of the next qc's loop, evacs stay off the busy ScalarE.
        for m in range(D // 128):
            yt = ps_a.tile([128, 1024], FP32, tag="big")
            for nl in range(2):
                n = n2 * 2 + nl
                nc.tensor.matmul(
                    yt[:, nl * 512:(nl + 1) * 512],
                    lhsT=wo_sb[:, m * 128:(m + 1) * 128],
                    rhs=oT_sb[0:HD, n * 512:(n + 1) * 512],
                    start=True,
                    stop=True,
                )
            y_sb = ypool.tile([128, 1024], FP16, tag="ysb")
            if vector_only or m % 2 == 0:
                nc.vector.tensor_copy(y_sb, yt)
            else:
                nc.scalar.copy(y_sb, yt)
            nc.sync.dma_start(
                out=ypT[m * 128:(m + 1) * 128, n2 * 1024:(n2 + 1) * 1024],
                in_=y_sb,
            )

    for qc in range(QC):
        ot = ps_o.tile([HD + 1, QL], FP32, tag="ot")
        pending = {}

        def emit_s(t, qc=qc, pending=pending):
            st = ps_a.tile([128, QL], FP32, tag="big")
            w_tile = wpool.tile([128, QL], FP16, tag="w")
            nc.sync.dma_start(
                out=w_tile,
                in_=wt[t * 128:(t + 1) * 128, qc * QL:(qc + 1) * QL],
            )
            for n in range(QL // 512):
                nc.tensor.matmul(
                    st[:, n * 512:(n + 1) * 512],
                    lhsT=kT_sb[:, t * 128:(t + 1) * 128],
                    rhs=qT_sb[:, qc * QL + n * 512: qc * QL + (n + 1) * 512],
                    start=True,
                    stop=True,
                )
            pending[t] = (st, w_tile)

        emit_s(0)
        emit_s(1)
        if qc == 0:
            # V' : [k-tile 128, 65] per tile, col 64 = 1.0. Emitted after the
            # first S tile so the first exp starts as early as possible.
            nc.vector.memset(v_sb, 1.0)
            vp = ps_a.tile([128, KT * HD], FP32, tag="big")
            for m in range(KT):
                for c in range(CH):
                    nc.tensor.matmul(
                        vp[:, m * HD:(m + 1) * HD],
                        lhsT=xT_cc[m // 4][:, c * 512 + (m % 4) * 128:
                                           c * 512 + (m % 4) * 128 + 128],
                        rhs=wv_sb[:, c * HD:(c + 1) * HD],
                        start=(c == 0),
                        stop=(c == CH - 1),
                    )
            nc.vector.tensor_copy(
                v_sb.rearrange("p (t c) -> p t c", c=HD + 1)[:, :, 0:HD],
                vp.rearrange("p (t c) -> p t c", c=HD),
            )
        for t in range(KT):
            if t + 2 < KT:
                emit_s(t + 2)
            st, w_tile = pending.pop(t)
            e_tile = epool.tile([128, QL], FP16, tag="e")
            nc.scalar.activation(e_tile, st, AF.Exp)
            p_tile = ppool.tile([128, QL], FP16, tag="p")
            nc.vector.tensor_mul(p_tile, e_tile, w_tile)
            for n in range(QL // 512):
                nc.tensor.matmul(
                    ot[:, n * 512:(n + 1) * 512],
                    lhsT=v_sb[:, t * (HD + 1):(t + 1) * (HD + 1)],
                    rhs=p_tile[:, n * 512:(n + 1) * 512],
                    start=(t == 0),
                    stop=(t == KT - 1),
                )
        nc.scalar.copy(oT_sb[:, qc * QL:(qc + 1) * QL], ot)

    emit_y(0, vector_only=False)
    emit_y(1, vector_only=False)
    nc.sync.dma_start(out=zrow.rearrange("(a n) -> a n", a=1),
                      in_=oT_sb[HD:HD + 1, :])

    # ---- tail: Y^T = Wo^T-tiles x O^T, evacs split across DVE/ACT -------



def _install_ntff_hook():
    """Recreate the missing ``antenv.axon_hooks`` module so that
    run_bass_kernel_spmd(trace=True) can capture NTFF profiles via the
    libaxon_pjrt.so ctypes hook (see trn_agent_boot.trn_boot)."""
    import sys
    import types

    try:
        import antenv.axon_hooks  # noqa: F401
        return
    except ImportError:
        pass
    import antenv
    from trn_agent_boot.trn_boot import _ntff_profile_via_ctypes

    mod = types.ModuleType("antenv.axon_hooks")
    mod._hook = _ntff_profile_via_ctypes("/opt/axon/libaxon_pjrt.so")
    mod.set_axon_ntff_profile_hook = lambda h: setattr(mod, "_hook", h)
    mod.get_axon_ntff_profile_hook = lambda: mod._hook
    sys.modules["antenv.axon_hooks"] = mod
    antenv.axon_hooks = mod
    # keep profile artifacts local; the sandbox has no bucket access
    bass_utils.upload_artifacts = lambda tmpdir: tmpdir


def kernel(x, z_matrix, Wq, bq, Wk, bk, Wv, bv, Wo, bo, z_table, _trace=False):
    if _trace:
        _install_ntff_hook()
    x = np.ascontiguousarray(np.asarray(x, dtype=np.float32))
    z_matrix = np.asarray(z_matrix, dtype=np.float32)
    Wq = np.asarray(Wq, dtype=np.float32)
    Wk = np.asarray(Wk, dtype=np.float32)
    Wv = np.asarray(Wv, dtype=np.float32)
    Wo = np.asarray(Wo, dtype=np.float32)
    bq = np.asarray(bq, dtype=np.float32)
    bk = np.asarray(bk, dtype=np.float32)
    bv = np.asarray(bv, dtype=np.float32)
    bo = np.asarray(bo, dtype=np.float32)
    z_table = np.asarray(z_table, dtype=np.float32)

    nc = _build_program()

    xT = np.ascontiguousarray(x.T).astype(BF16_NP)
    binsT = np.clip(
        np.floor(z_matrix.T / MAX_Z * NUM_Z_BINS).astype(np.int32), 0, NUM_Z_BINS - 1
    )
    exp_tab = np.exp(z_table)  # [16, H] fp32
    sclv = np.concatenate([np.full(HD, SCALE, np.float32),
                           np.ones(HD, np.float32)])

    in_maps = []
    for h in range(NCORES):
        sl = slice(h * HD, (h + 1) * HD)
        wt_h = exp_tab[:, h][binsT].astype(FP16_NP)  # [key, query] layout
        in_maps.append({
            "xT": xT,
            "wqk": np.ascontiguousarray(
                np.concatenate([Wq[:, sl], Wk[:, sl]], axis=1)).astype(BF16_NP),
            "wv": np.ascontiguousarray(Wv[:, sl]).astype(BF16_NP),
            "wo": np.ascontiguousarray(Wo[sl, :]).astype(FP16_NP),
            "bqk": np.concatenate([bq[sl], bk[sl]]),
            "sclv": sclv,
            "wt": wt_h,
        })

    res = bass_utils.run_bass_kernel_spmd(
        nc, in_maps, core_ids=list(range(NCORES)), trace=_trace,
    )

    acc = np.zeros((D, N), dtype=np.float64)
    for h in range(NCORES):
        ypT_h = res.results[h]["ypT"].astype(np.float64)
        z_h = res.results[h]["zrow"].astype(np.float64)
        acc += ypT_h / z_h[None, :]
    out = acc.T + (bv @ Wo)[None, :] + bo[None, :]
    out_f32 = out.astype(np.float32)
    if _trace:
        return out_f32, res
    return out_f32



# revision 11
# speedup vs baseline: 1.0928x; 1.0928x over previous
"""Graphormer attention (N=2048, D=512, H=8 heads of 64) on 8 NeuronCores.

Strategy (tensor-parallel over heads, one head per core):
  - Host slices Q/K/V/O projection weights per head, transposes x once.
  - The z-bin bias is folded in multiplicatively: the per-head bias table is
    tiny (16 entries), so host precomputes W = exp(z_table[bin(z)]) transposed
    to the kernel's [key, query] layout, shipped as fp16.
  - On device (per core): fused Q^T/K^T projection (softmax scale folded into
    Wq on host), V projection, then a software-pipelined loop over key tiles:
    S^T = K^T-tiles x Q^T (fp32, PSUM), exp on ScalarE -> fp16,
    P = exp(S) * W on VectorE, O'^T = sum_k V'[k,65] x P (65th V column = 1
    => row 64 of O' is the softmax denominator Z), Y^T = Wo_h^T-tiles x O^T.
  - The loop cadence is bound by ScalarE exp ([128,1024] ~1.15us each); all
    matmuls, DMA and DVE multiplies hide underneath it.  Prologue evacuations
    (QK/V PSUM->SBUF) are spliced between the first few DVE multiplies, and
    Y^T for query chunk 0 is interleaved into chunk 1's loop.
  - Host divides each head's partial Y by its Z, sums heads, adds bias terms.
"""

import numpy as np
import ml_dtypes
from contextlib import ExitStack

import concourse.bass as bass
import concourse.tile as tile
from concourse import bacc, mybir
from concourse import bass_utils

N = 2048
D = 512
H = 8
HD = 64
NUM_Z_BINS = 16
MAX_Z = 5.0
SCALE = HD ** -0.5
NCORES = 8
QL = 1024          # query-chunk length (PSUM budget)
QC = N // QL       # 2 query chunks
KT = N // 128      # 16 key tiles

FP32 = mybir.dt.float32
FP16 = mybir.dt.float16
BF16 = mybir.dt.bfloat16
BF16_NP = ml_dtypes.bfloat16
FP16_NP = np.float16

AF = mybir.ActivationFunctionType
OP = mybir.AluOpType

_PROGRAM_CACHE = {}


def _build_program():
    if "nc" in _PROGRAM_CACHE:
        return _PROGRAM_CACHE["nc"]

    nc = bacc.Bacc(
        "TRN2",
        target_bir_lowering=False,
        debug=False,
        enable_asserts=False,
        num_devices=NCORES,
    )

    xT = nc.dram_tensor("xT", [D, N], BF16, kind="ExternalInput").ap()
    wqk = nc.dram_tensor("wqk", [D, 128], BF16, kind="ExternalInput").ap()
    wv = nc.dram_tensor("wv", [D, HD], BF16, kind="ExternalInput").ap()
    wo = nc.dram_tensor("wo", [HD, D], FP16, kind="ExternalInput").ap()
    wt = nc.dram_tensor("wt", [N, N], FP16, kind="ExternalInput").ap()

    ypT = nc.dram_tensor("ypT", [D, N], FP16, kind="ExternalOutput").ap()
    zrow = nc.dram_tensor("zrow", [N], FP16, kind="ExternalOutput").ap()

    with tile.TileContext(nc) as tc:
        with ExitStack() as ctx:
            _emit(ctx, tc, xT, wqk, wv, wo, wt, ypT, zrow)
    nc.compile()
    _PROGRAM_CACHE["nc"] = nc
    return nc


def _emit(ctx, tc, xT, wqk, wv, wo, wt, ypT, zrow):
    nc = tc.nc
    CH = D // 128  # 4 contraction chunks of the model dim

    singles = ctx.enter_context(tc.tile_pool(name="singles", bufs=1))
    # PSUM budget is 16KB/partition (8 banks). ps_a slots are [128,1024]fp32
    # (2 banks x 3 slots = 6 banks) shared by the qk-proj/S/Y phases; ps_o
    # (2 banks) holds the V-projection scratch then the O' accumulator.
    ps_a = ctx.enter_context(tc.tile_pool(name="ps_a", bufs=3, space="PSUM"))
    ps_o = ctx.enter_context(tc.tile_pool(name="ps_o", bufs=1, space="PSUM"))
    wpool = ctx.enter_context(tc.tile_pool(name="wpool", bufs=6))
    epool = ctx.enter_context(tc.tile_pool(name="epool", bufs=8))
    ppool = ctx.enter_context(tc.tile_pool(name="ppool", bufs=8))
    ypool = ctx.enter_context(tc.tile_pool(name="ypool", bufs=4))

    # ---- constants + x^T load ------------------------------------------
    # wqk first (needed by the first QK matmul), then x^T query-half 0
    # (chunked by contraction c, full 2KB DMA lines), then the first two
    # W tiles, then wv + x^T half 1 + wo.  S-tiles for keys 0..1023 and
    # query chunk 0 need only half 0, so compute starts ~3us in.
    wqk_sb = singles.tile([128, CH * 128], BF16)
    for c in range(CH):
        nc.sync.dma_start(out=wqk_sb[:, c * 128:(c + 1) * 128],
                          in_=wqk.rearrange("(c p) m -> c p m", p=128)[c])

    xr = xT.rearrange("(c p) (h n) -> c h p n", p=128, n=QL)
    xT_sb = []
    for c in range(CH):
        t_ = singles.tile([128, N], BF16, tag=f"xc{c}")
        xT_sb.append(t_)
    for c in range(CH):
        nc.sync.dma_start(out=xT_sb[c][:, 0:QL], in_=xr[c, 0])

    w_tiles = {}

    def issue_w(qc, t):
        w_tile = wpool.tile([128, QL], FP16, tag="w")
        nc.sync.dma_start(
            out=w_tile,
            in_=wt[t * 128:(t + 1) * 128, qc * QL:(qc + 1) * QL],
        )
        w_tiles[(qc, t)] = w_tile

    issue_w(0, 0)
    issue_w(0, 1)

    wv_sb = singles.tile([128, CH * HD], BF16)
    for c in range(CH):
        nc.sync.dma_start(out=wv_sb[:, c * HD:(c + 1) * HD],
                          in_=wv.rearrange("(c p) m -> c p m", p=128)[c])
    for c in range(CH):
        nc.sync.dma_start(out=xT_sb[c][:, QL:N], in_=xr[c, 1])
    wo_sb = singles.tile([HD, D], FP16)
    nc.sync.dma_start(out=wo_sb, in_=wo)
    issue_w(0, 2)
    issue_w(0, 3)

    # ---- PE warm-up: dummy matmuls ramp the p-state while DMA streams --
    scratch = singles.tile([128, 512], BF16)
    nc.vector.memset(scratch, 0.0)
    wu = ps_a.tile([128, QL], FP32, tag="big")
    for _ in range(3):
        nc.tensor.matmul(wu[:, 0:512], lhsT=scratch[:, 0:128], rhs=scratch,
                         start=True, stop=True)

    # ---- fused Q^T/K^T projection: one [128,128] weight block computes
    # (scaled) Q^T into PSUM rows 0-63 and K^T into rows 64-127.  The
    # matmul requires both operands at the same base partition, so the
    # evacuation splits into separate q/k tiles (Q on ACT while it's idle
    # pre-exp, the rest on DVE).
    qT_sb = singles.tile([HD, N], BF16)
    kT_sb = singles.tile([HD, N], BF16)
    qk_pending = {}

    def emit_qk_mm(jp):
        pt = ps_a.tile([128, QL], FP32, tag="big")
        for jj in range(2):
            lo = jp * QL + jj * 512
            for c in range(CH):
                nc.tensor.matmul(
                    pt[:, jj * 512:(jj + 1) * 512],
                    lhsT=wqk_sb[:, c * 128:(c + 1) * 128],
                    rhs=xT_sb[c][:, lo:lo + 512],
                    start=(c == 0),
                    stop=(c == CH - 1),
                )
        qk_pending[jp] = pt

    def evac_qk(jp, q_on_act):
        pt = qk_pending.pop(jp)
        dst = slice(jp * QL, (jp + 1) * QL)
        if q_on_act:
            nc.scalar.copy(qT_sb[:, dst], pt[0:HD, :])
        else:
            nc.vector.tensor_copy(qT_sb[:, dst], pt[0:HD, :])
        nc.vector.tensor_copy(kT_sb[:, dst], pt[HD:128, :])

    # ---- S tile emission ------------------------------------------------
    pending = {}

    def emit_s(qc, t):
        st = ps_a.tile([128, QL], FP32, tag="big")
        for n in range(QL // 512):
            nc.tensor.matmul(
                st[:, n * 512:(n + 1) * 512],
                lhsT=kT_sb[:, t * 128:(t + 1) * 128],
                rhs=qT_sb[:, qc * QL + n * 512: qc * QL + (n + 1) * 512],
                start=True,
                stop=True,
            )
        pending[(qc, t)] = st

    # ---- V projection: V' = [k-tile 128, 65] per tile, col 64 = 1.0 ----
    v_sb = singles.tile([128, KT * (HD + 1)], FP16)
    nc.vector.memset(v_sb, 1.0)
    v_pending = {}

    def emit_v_mm(half):
        vp = ps_o.tile([128, QL], FP32, tag="ot")
        for mm in range(KT // 2):
            m = half * (KT // 2) + mm
            for c in range(CH):
                nc.tensor.matmul(
                    vp[:, mm * HD:(mm + 1) * HD],
                    lhsT=xT_sb[c][:, m * 128:(m + 1) * 128],
                    rhs=wv_sb[:, c * HD:(c + 1) * HD],
                    start=(c == 0),
                    stop=(c == CH - 1),
                )
        v_pending[half] = vp

    def evac_v(half):
        mlo = half * (KT // 2)
        vp = v_pending.pop(half)
        nc.vector.tensor_copy(
            v_sb.rearrange("p (t c) -> p t c", c=HD + 1)
                [:, mlo:mlo + KT // 2, 0:HD],
            vp[:, 0:KT // 2 * HD].rearrange("p (t c) -> p t c", c=HD),
        )

    # PE order: warmup, QK(0), S0, S1, QK(1), S2, S3, V(0), V(1), loop.
    # The prologue evacuations queue up on DVE ahead of the multiplies;
    # the deep e/p pools let the exp cadence run while DVE drains them.
    emit_qk_mm(0)
    evac_qk(0, q_on_act=True)
    emit_s(0, 0)
    emit_s(0, 1)
    emit_qk_mm(1)
    evac_qk(1, q_on_act=False)
    emit_s(0, 2)
    emit_s(0, 3)
    emit_v_mm(0)
    evac_v(0)
    emit_v_mm(1)
    evac_v(1)

    oT_sb = singles.tile([HD + 1, N], FP16)

    def emit_y(n2, m, tail):
        # Y^T block for query columns [n2*1024, (n2+1)*1024), model rows
        # [m*128, (m+1)*128).  Evacuations go to DVE mid-loop (ACT is the
        # cadence-critical engine); tail blocks alternate ACT/DVE.
        yt = ps_a.tile([128, QL], FP32, tag="big")
        for nl in range(2):
            n = n2 * 2 + nl
            nc.tensor.matmul(
                yt[:, nl * 512:(nl + 1) * 512],
                lhsT=wo_sb[:, m * 128:(m + 1) * 128],
                rhs=oT_sb[0:HD, n * 512:(n + 1) * 512],
                start=True,
                stop=True,
            )
        y_sb = ypool.tile([128, QL], FP16, tag="ysb")
        if tail and m % 2 == 1:
            nc.scalar.copy(y_sb, yt)
        else:
            nc.vector.tensor_copy(y_sb, yt)
        nc.sync.dma_start(
            out=ypT[m * 128:(m + 1) * 128, n2 * QL:(n2 + 1) * QL],
            in_=y_sb,
        )

    # ---- main loop: exp -> *W -> PV with S(t+4)/W(t+4) prefetch ---------
    for qc in range(QC):
        ot = ps_o.tile([HD + 1, QL], FP32, tag="ot")
        for t in range(KT):
            gt = qc * KT + t          # global tile index 0..31
            ta = gt + 4
            if ta < QC * KT:
                emit_s(ta // KT, ta % KT)
            tw = gt + 4
            if tw < QC * KT:
                issue_w(tw // KT, tw % KT)
            st = pending.pop((qc, t))
            w_tile = w_tiles.pop((qc, t))
            e_tile = epool.tile([128, QL], FP16, tag="e")
            nc.scalar.activation(e_tile, st, AF.Exp)
            p_tile = ppool.tile([128, QL], FP16, tag="p")
            nc.vector.tensor_mul(p_tile, e_tile, w_tile)
            for n in range(QL // 512):
                nc.tensor.matmul(
                    ot[:, n * 512:(n + 1) * 512],
                    lhsT=v_sb[:, t * (HD + 1):(t + 1) * (HD + 1)],
                    rhs=p_tile[:, n * 512:(n + 1) * 512],
                    start=(t == 0),
                    stop=(t == KT - 1),
                )
            if qc == 1 and t in (3, 6, 9, 12):
                emit_y(0, t // 3 - 1, tail=False)
        nc.vector.tensor_copy(oT_sb[:, qc * QL:(qc + 1) * QL], ot)

    for m in range(D // 128):
        emit_y(1, m, tail=True)
    nc.sync.dma_start(out=zrow.rearrange("(a n) -> a n", a=1),
                      in_=oT_sb[HD:HD + 1, :])


def _install_ntff_hook():
    """Recreate the missing ``antenv.axon_hooks`` module so that
    run_bass_kernel_spmd(trace=True) can capture NTFF profiles via the
    libaxon_pjrt.so ctypes hook (see trn_agent_boot.trn_boot)."""
    import sys
    import types

    try:
        import antenv.axon_hooks  # noqa: F401
        return
    except ImportError:
        pass
    import antenv
    from trn_agent_boot.trn_boot import _ntff_profile_via_ctypes

    mod = types.ModuleType("antenv.axon_hooks")
    mod._hook = _ntff_profile_via_ctypes("/opt/axon/libaxon_pjrt.so")
    mod.set_axon_ntff_profile_hook = lambda h: setattr(mod, "_hook", h)
    mod.get_axon_ntff_profile_hook = lambda: mod._hook
    sys.modules["antenv.axon_hooks"] = mod
    antenv.axon_hooks = mod
    # keep profile artifacts local; the sandbox has no bucket access
    bass_utils.upload_artifacts = lambda tmpdir: tmpdir


def kernel(x, z_matrix, Wq, bq, Wk, bk, Wv, bv, Wo, bo, z_table, _trace=False):
    if _trace:
        _install_ntff_hook()
    x = np.ascontiguousarray(np.asarray(x, dtype=np.float32))
    z_matrix = np.asarray(z_matrix, dtype=np.float32)
    Wq = np.asarray(Wq, dtype=np.float32)
    Wk = np.asarray(Wk, dtype=np.float32)
    Wv = np.asarray(Wv, dtype=np.float32)
    Wo = np.asarray(Wo, dtype=np.float32)
    bq = np.asarray(bq, dtype=np.float32)
    bk = np.asarray(bk, dtype=np.float32)
    bv = np.asarray(bv, dtype=np.float32)
    bo = np.asarray(bo, dtype=np.float32)
    z_table = np.asarray(z_table, dtype=np.float32)

    nc = _build_program()

    xT = np.ascontiguousarray(x.T).astype(BF16_NP)
    binsT = np.clip(
        np.floor(z_matrix.T / MAX_Z * NUM_Z_BINS).astype(np.int32), 0, NUM_Z_BINS - 1
    )
    exp_tab = np.exp(z_table)  # [16, H] fp32

    in_maps = []
    for h in range(NCORES):
        sl = slice(h * HD, (h + 1) * HD)
        wt_h = exp_tab[:, h][binsT]  # [key, query] layout
        if bq[sl].any() or bk[sl].any():
            # logits = scale*(q+bq).(k+bk); per-query terms cancel in
            # softmax, leaving a per-key multiplicative factor.
            key_term = SCALE * ((x @ Wk[:, sl] + bk[sl]) @ bq[sl])  # [N]
            wt_h = wt_h * np.exp(key_term)[:, None]
        wt_h = wt_h.astype(FP16_NP)
        in_maps.append({
            "xT": xT,
            "wqk": np.ascontiguousarray(
                np.concatenate([Wq[:, sl] * SCALE, Wk[:, sl]], axis=1)
            ).astype(BF16_NP),
            "wv": np.ascontiguousarray(Wv[:, sl]).astype(BF16_NP),
            "wo": np.ascontiguousarray(Wo[sl, :]).astype(FP16_NP),
            "wt": wt_h,
        })

    res = bass_utils.run_bass_kernel_spmd(
        nc, in_maps, core_ids=list(range(NCORES)), trace=_trace,
    )

    acc = np.zeros((D, N), dtype=np.float64)
    for h in range(NCORES):
        ypT_h = res.results[h]["ypT"].astype(np.float64)
        z_h = res.results[h]["zrow"].astype(np.float64)
        acc += ypT_h / z_h[None, :]
    out = acc.T + (bv @ Wo)[None, :] + bo[None, :]
    out_f32 = out.astype(np.float32)
    if _trace:
        return out_f32, res
    return out_f32


# revision 19
# speedup vs baseline: 1.1291x; 1.0332x over previous
"""Graphormer attention (N=2048, D=512, H=8 heads of 64) on 8 NeuronCores.

Strategy (tensor-parallel over heads, one head per core):
  - Host slices Q/K/V/O projection weights per head, transposes x once.
  - The z-bin bias is folded in multiplicatively: the per-head bias table is
    tiny (16 entries), so host precomputes W = exp(z_table[bin(z)]) transposed
    to the kernel's [key, query] layout, shipped as fp16.
  - On device (per core): fused Q^T/K^T projection (softmax scale folded into
    Wq on host), V projection, then a software-pipelined loop over key tiles:
    S^T = K^T-tiles x Q^T (fp32, PSUM), exp on ScalarE -> fp16,
    P = exp(S) * W on VectorE, O'^T = sum_k V'[k,65] x P (65th V column = 1
    => row 64 of O' is the softmax denominator Z), Y^T = Wo_h^T-tiles x O^T.
  - The loop cadence is bound by ScalarE exp ([128,1024] ~1.15us each); all
    matmuls, DMA and DVE multiplies hide underneath it.  Prologue evacuations
    (QK/V PSUM->SBUF) are spliced between the first few DVE multiplies, and
    Y^T for query chunk 0 is interleaved into chunk 1's loop.
  - Host divides each head's partial Y by its Z, sums heads, adds bias terms.
"""

import numpy as np
import ml_dtypes
from contextlib import ExitStack

import concourse.bass as bass
import concourse.tile as tile
from concourse import bacc, mybir
from concourse import bass_utils

N = 2048
D = 512
H = 8
HD = 64
NUM_Z_BINS = 16
MAX_Z = 5.0
SCALE = HD ** -0.5
NCORES = 8
QL = 1024          # query-chunk length (PSUM budget)
QC = N // QL       # 2 query chunks
KT = N // 128      # 16 key tiles

FP32 = mybir.dt.float32
FP16 = mybir.dt.float16
BF16 = mybir.dt.bfloat16
BF16_NP = ml_dtypes.bfloat16
FP16_NP = np.float16

AF = mybir.ActivationFunctionType
OP = mybir.AluOpType

_PROGRAM_CACHE = {}


def _build_program():
    if "nc" in _PROGRAM_CACHE:
        return _PROGRAM_CACHE["nc"]

    nc = bacc.Bacc(
        "TRN2",
        target_bir_lowering=False,
        debug=False,
        enable_asserts=False,
        num_devices=NCORES,
    )

    # host-packed layouts so every DMA has >=4KB contiguous lines:
    #   xT:  [128, (h=2, c=4, n=1024)] -- x^T chunked by query half/contraction
    #   wqv: [128, (c=4)*128 qk | (c=4)*64 v] -- per-head projection weights
    #   wt:  [(qc=2, tp=8)*128, 2048] -- bias tiles in key-tile pairs
    xT = nc.dram_tensor("xT", [128, 2 * D // 128 * QL], BF16,
                        kind="ExternalInput").ap()
    wqv = nc.dram_tensor("wqv", [128, D + D // 2], BF16,
                         kind="ExternalInput").ap()
    wo = nc.dram_tensor("wo", [HD, D], FP16, kind="ExternalInput").ap()
    wt = nc.dram_tensor("wt", [N, N], FP16, kind="ExternalInput").ap()

    ypT = nc.dram_tensor("ypT", [D, N], FP16, kind="ExternalOutput").ap()
    zrow = nc.dram_tensor("zrow", [N], FP16, kind="ExternalOutput").ap()

    with tile.TileContext(nc) as tc:
        with ExitStack() as ctx:
            _emit(ctx, tc, xT, wqv, wo, wt, ypT, zrow)
    nc.compile()
    _PROGRAM_CACHE["nc"] = nc
    return nc


def _emit(ctx, tc, xT, wqv, wo, wt, ypT, zrow):
    nc = tc.nc
    CH = D // 128  # 4 contraction chunks of the model dim

    singles = ctx.enter_context(tc.tile_pool(name="singles", bufs=1))
    # PSUM budget is 16KB/partition (8 banks). ps_a slots are [128,1024]fp32
    # (2 banks x 3 slots = 6 banks) shared by the qk-proj/S/Y phases; ps_o
    # (2 banks) holds the V-projection scratch then the O' accumulator.
    ps_a = ctx.enter_context(tc.tile_pool(name="ps_a", bufs=3, space="PSUM"))
    ps_o = ctx.enter_context(tc.tile_pool(name="ps_o", bufs=1, space="PSUM"))
    wpool = ctx.enter_context(tc.tile_pool(name="wpool", bufs=4))
    epool = ctx.enter_context(tc.tile_pool(name="epool", bufs=8))
    ppool = ctx.enter_context(tc.tile_pool(name="ppool", bufs=8))
    ypool = ctx.enter_context(tc.tile_pool(name="ypool", bufs=4))

    # ---- constants + x^T load ------------------------------------------
    # wqv first (needed by the first QK matmul), then x^T in 512KB pieces
    # (8KB lines), with the first W pair spliced before x half 1.  S-tiles
    # of keys 0..1023 / query chunk 0 need only half 0.
    wqv_sb = singles.tile([128, D + D // 2], BF16)
    nc.sync.dma_start(out=wqv_sb, in_=wqv)
    wqk_sb = wqv_sb[:, 0:D]
    wv_sb = wqv_sb[:, D:D + D // 2]

    xT_sb = singles.tile([128, 2 * CH * QL], BF16)

    def xs(g0, c, width):
        # x^T chunk c, global query/key columns [g0, g0+width) (one h-half)
        h, off = divmod(g0, QL)
        base = (h * CH + c) * QL + off
        return xT_sb[:, base:base + width]

    def load_x(h, cp):
        lo = (h * CH + 2 * cp) * QL
        nc.sync.dma_start(out=xT_sb[:, lo:lo + 2 * QL],
                          in_=xT[:, lo:lo + 2 * QL])

    load_x(0, 0)
    load_x(0, 1)

    w_pairs = {}

    def issue_w(qc, tp):
        w_tile = wpool.tile([128, 2 * QL], FP16, tag="w")
        nc.sync.dma_start(
            out=w_tile,
            in_=wt[(qc * 8 + tp) * 128:(qc * 8 + tp + 1) * 128, :],
        )
        w_pairs[(qc, tp)] = w_tile

    issue_w(0, 0)
    load_x(1, 0)
    load_x(1, 1)
    wo_sb = singles.tile([HD, D], FP16)
    nc.sync.dma_start(out=wo_sb, in_=wo)
    issue_w(0, 1)
    issue_w(0, 2)

    # ---- PE warm-up: dummy matmuls ramp the p-state while DMA streams --
    scratch = singles.tile([128, 512], BF16)
    nc.vector.memset(scratch, 0.0)
    wu = ps_a.tile([128, QL], FP32, tag="big")
    for _ in range(7):
        nc.tensor.matmul(wu[:, 0:512], lhsT=scratch[:, 0:128], rhs=scratch,
                         start=True, stop=True)

    # ---- fused Q^T/K^T projection: one [128,128] weight block computes
    # (scaled) Q^T into PSUM rows 0-63 and K^T into rows 64-127.  The
    # matmul requires both operands at the same base partition, so the
    # evacuation splits into separate q/k tiles (Q on ACT while it's idle
    # pre-exp, the rest on DVE).
    qT_sb = singles.tile([HD, N], BF16)
    kT_sb = singles.tile([HD, N], BF16)
    qk_pending = {}

    def emit_qk_mm(jp):
        # c-major so each matmul is gated only by its own x chunk's DMA
        pt = ps_a.tile([128, QL], FP32, tag="big")
        for c in range(CH):
            for jj in range(2):
                nc.tensor.matmul(
                    pt[:, jj * 512:(jj + 1) * 512],
                    lhsT=wqk_sb[:, c * 128:(c + 1) * 128],
                    rhs=xs(jp * QL + jj * 512, c, 512),
                    start=(c == 0),
                    stop=(c == CH - 1),
                )
        qk_pending[jp] = pt

    def evac_qk(jp, q_on_act):
        pt = qk_pending.pop(jp)
        dst = slice(jp * QL, (jp + 1) * QL)
        if q_on_act:
            nc.scalar.copy(qT_sb[:, dst], pt[0:HD, :])
        else:
            nc.vector.tensor_copy(qT_sb[:, dst], pt[0:HD, :])
        nc.vector.tensor_copy(kT_sb[:, dst], pt[HD:128, :])

    # ---- S tile emission ------------------------------------------------
    pending = {}

    def emit_s(qc, t):
        st = ps_a.tile([128, QL], FP32, tag="big")
        for n in range(QL // 512):
            nc.tensor.matmul(
                st[:, n * 512:(n + 1) * 512],
                lhsT=kT_sb[:, t * 128:(t + 1) * 128],
                rhs=qT_sb[:, qc * QL + n * 512: qc * QL + (n + 1) * 512],
                start=True,
                stop=True,
            )
        pending[(qc, t)] = st

    # ---- V projection: V' = [k-tile 128, 65] per tile, col 64 = 1.0 ----
    v_sb = singles.tile([128, KT * (HD + 1)], FP16)
    nc.vector.memset(v_sb, 1.0)
    v_pending = {}

    def emit_v_mm(half):
        vp = ps_o.tile([128, QL], FP32, tag="ot")
        for mm in range(KT // 2):
            m = half * (KT // 2) + mm
            for c in range(CH):
                nc.tensor.matmul(
                    vp[:, mm * HD:(mm + 1) * HD],
                    lhsT=xs(m * 128, c, 128),
                    rhs=wv_sb[:, c * HD:(c + 1) * HD],
                    start=(c == 0),
                    stop=(c == CH - 1),
                )
        v_pending[half] = vp

    def evac_v(half):
        mlo = half * (KT // 2)
        vp = v_pending.pop(half)
        nc.vector.tensor_copy(
            v_sb.rearrange("p (t c) -> p t c", c=HD + 1)
                [:, mlo:mlo + KT // 2, 0:HD],
            vp[:, 0:KT // 2 * HD].rearrange("p (t c) -> p t c", c=HD),
        )

    # PE order: warmup, QK(0), S0..S3, QK(1), V(0), V(1), loop.  S4..S7
    # (emitted in the loop) still only need query-half-0 data.  The
    # prologue evacuations queue up on DVE ahead of the multiplies; the
    # deep e/p pools let the exp cadence run while DVE drains them.
    emit_qk_mm(0)
    evac_qk(0, q_on_act=True)
    emit_s(0, 0)
    emit_s(0, 1)
    emit_s(0, 2)
    emit_s(0, 3)
    emit_qk_mm(1)
    evac_qk(1, q_on_act=False)
    emit_v_mm(0)
    evac_v(0)
    emit_v_mm(1)
    evac_v(1)

    oT_sb = singles.tile([HD + 1, N], FP16)

    def emit_y(n2, m, tail):
        # Y^T block for query columns [n2*1024, (n2+1)*1024), model rows
        # [m*128, (m+1)*128).  Evacuations go to DVE mid-loop (ACT is the
        # cadence-critical engine); tail blocks alternate ACT/DVE.
        yt = ps_a.tile([128, QL], FP32, tag="big")
        for nl in range(2):
            n = n2 * 2 + nl
            nc.tensor.matmul(
                yt[:, nl * 512:(nl + 1) * 512],
                lhsT=wo_sb[:, m * 128:(m + 1) * 128],
                rhs=oT_sb[0:HD, n * 512:(n + 1) * 512],
                start=True,
                stop=True,
            )
        y_sb = ypool.tile([128, QL], FP16, tag="ysb")
        if tail and m % 2 == 0:
            nc.scalar.copy(y_sb, yt)
        else:
            nc.vector.tensor_copy(y_sb, yt)
        nc.sync.dma_start(
            out=ypT[m * 128:(m + 1) * 128, n2 * QL:(n2 + 1) * QL],
            in_=y_sb,
        )

    # ---- main loop: exp -> *W -> PV with S(t+4)/W-pair(+3) prefetch -----
    for qc in range(QC):
        ot = ps_o.tile([HD + 1, QL], FP32, tag="ot")
        for t in range(KT):
            gt = qc * KT + t          # global tile index 0..31
            ta = gt + 4
            if ta < QC * KT:
                emit_s(ta // KT, ta % KT)
            if gt % 2 == 0:
                pp = gt // 2 + 3
                if pp < QC * KT // 2:
                    issue_w(pp // 8, pp % 8)
            st = pending.pop((qc, t))
            wp = w_pairs[(qc, t // 2)]
            e_tile = epool.tile([128, QL], FP16, tag="e")
            nc.scalar.activation(e_tile, st, AF.Exp)
            p_tile = ppool.tile([128, QL], FP16, tag="p")
            nc.vector.tensor_mul(p_tile, e_tile,
                                 wp[:, (t % 2) * QL:(t % 2 + 1) * QL])
            if t % 2 == 1:
                del w_pairs[(qc, t // 2)]
            for n in range(QL // 512):
                nc.tensor.matmul(
                    ot[:, n * 512:(n + 1) * 512],
                    lhsT=v_sb[:, t * (HD + 1):(t + 1) * (HD + 1)],
                    rhs=p_tile[:, n * 512:(n + 1) * 512],
                    start=(t == 0),
                    stop=(t == KT - 1),
                )
            if qc == 1 and t in (3, 6, 9, 12):
                emit_y(0, t // 3 - 1, tail=False)
        nc.vector.tensor_copy(oT_sb[:, qc * QL:(qc + 1) * QL], ot)

    for m in range(D // 128):
        emit_y(1, m, tail=True)
    nc.sync.dma_start(out=zrow.rearrange("(a n) -> a n", a=1),
                      in_=oT_sb[HD:HD + 1, :])


def _install_ntff_hook():
    """Recreate the missing ``antenv.axon_hooks`` module so that
    run_bass_kernel_spmd(trace=True) can capture NTFF profiles via the
    libaxon_pjrt.so ctypes hook (see trn_agent_boot.trn_boot)."""
    import sys
    import types

    try:
        import antenv.axon_hooks  # noqa: F401
        return
    except ImportError:
        pass
    import antenv
    from trn_agent_boot.trn_boot import _ntff_profile_via_ctypes

    mod = types.ModuleType("antenv.axon_hooks")
    mod._hook = _ntff_profile_via_ctypes("/opt/axon/libaxon_pjrt.so")
    mod.set_axon_ntff_profile_hook = lambda h: setattr(mod, "_hook", h)
    mod.get_axon_ntff_profile_hook = lambda: mod._hook
    sys.modules["antenv.axon_hooks"] = mod
    antenv.axon_hooks = mod
    # keep profile artifacts local; the sandbox has no bucket access
    bass_utils.upload_artifacts = lambda tmpdir: tmpdir


def kernel(x, z_matrix, Wq, bq, Wk, bk, Wv, bv, Wo, bo, z_table, _trace=False):
    if _trace:
        _install_ntff_hook()
    x = np.ascontiguousarray(np.asarray(x, dtype=np.float32))
    z_matrix = np.asarray(z_matrix, dtype=np.float32)
    Wq = np.asarray(Wq, dtype=np.float32)
    Wk = np.asarray(Wk, dtype=np.float32)
    Wv = np.asarray(Wv, dtype=np.float32)
    Wo = np.asarray(Wo, dtype=np.float32)
    bq = np.asarray(bq, dtype=np.float32)
    bk = np.asarray(bk, dtype=np.float32)
    bv = np.asarray(bv, dtype=np.float32)
    bo = np.asarray(bo, dtype=np.float32)
    z_table = np.asarray(z_table, dtype=np.float32)

    nc = _build_program()

    # pack x^T as [128, (h, c, n)] so each DMA has 8KB contiguous lines
    xTp = np.ascontiguousarray(
        x.T.reshape(4, 128, 2, 1024).transpose(1, 2, 0, 3).reshape(128, 8192)
    ).astype(BF16_NP)
    binsT = np.clip(
        np.floor(z_matrix.T / MAX_Z * NUM_Z_BINS).astype(np.int32), 0, NUM_Z_BINS - 1
    )
    exp_tab = np.exp(z_table)  # [16, H] fp32

    in_maps = []
    for h in range(NCORES):
        sl = slice(h * HD, (h + 1) * HD)
        wt_h = exp_tab[:, h][binsT]  # [key, query] layout
        if bq[sl].any() or bk[sl].any():
            # logits = scale*(q+bq).(k+bk); per-query terms cancel in
            # softmax, leaving a per-key multiplicative factor.
            key_term = SCALE * ((x @ Wk[:, sl] + bk[sl]) @ bq[sl])  # [N]
            wt_h = wt_h * np.exp(key_term)[:, None]
        # key-tile pairs: [(qc, tp)*128, (tl, n)] with 4KB lines
        wt_h = np.ascontiguousarray(
            wt_h.reshape(8, 2, 128, 2, 1024).transpose(3, 0, 2, 1, 4)
            .reshape(2048, 2048)
        ).astype(FP16_NP)
        wqk_h = np.concatenate([Wq[:, sl] * SCALE, Wk[:, sl]], axis=1)
        wqv_h = np.concatenate([
            wqk_h.reshape(4, 128, 128).transpose(1, 0, 2).reshape(128, 512),
            Wv[:, sl].reshape(4, 128, 64).transpose(1, 0, 2).reshape(128, 256),
        ], axis=1)
        in_maps.append({
            "xT": xTp,
            "wqv": np.ascontiguousarray(wqv_h).astype(BF16_NP),
            "wo": np.ascontiguousarray(Wo[sl, :]).astype(FP16_NP),
            "wt": wt_h,
        })

    res = bass_utils.run_bass_kernel_spmd(
        nc, in_maps, core_ids=list(range(NCORES)), trace=_trace,
    )

    acc = np.zeros((D, N), dtype=np.float64)
    for h in range(NCORES):
        ypT_h = res.results[h]["ypT"].astype(np.float64)
        z_h = res.results[h]["zrow"].astype(np.float64)
        acc += ypT_h / z_h[None, :]
    out = acc.T + (bv @ Wo)[None, :] + bo[None, :]
    out_f32 = out.astype(np.float32)
    if _trace:
        return out_f32, res
    return out_f32
